# revision 1
# baseline (speedup 1.0000x reference)
"""Trainium2 Bass kernel for the tree-structured dependency encoder.

Reference semantics (per node i, children-first topological order):
    leaf:     z_i = x_i
    internal: mult = max_c params[dep_c] * relu(z_{child_c})[None, :]   # [D, D]
              z_i  = x_i @ mult                                          # [D]
Output: z_root (root = node N-1), shape [1, D].

Strategy
--------
Only the root's ancestor cone matters (z flows child -> parent only), so the
host prunes the graph to nodes reachable from the root (~35 of 256 for the
reference tree) and dedupes (child, dep) edges.

Column sharding across the 8 cores: cand[:, j] = params[d][:, j] * relu(z_c[j])
depends only on column j of the child z, and z_i[j] = x_i @ mult[:, j] needs
the full x_i (an input) plus column j of mult.  So core k owns columns
[128k, 128k+128) of every z with ZERO cross-core communication; the host
concatenates the 8 root shards at the end.

Per-core layout ("colT"): each needed dep matrix is stored [128 part = j,
D free = i'] so the per-child scale relu(z_c[j]) is a per-partition scalar.
Needed-label params are DMA'd to SBUF once (first-use order) and reused.

Op mapping (HW-microbenchmarked; this walrus runs fused STT at ~1.2us but
TS-scalar bf16 at ~0.4us and TT bf16 at ~0.6us):
  mult   t_e = p_d * s_c   DVE tensor_scalar (0.4us) / ACT mul (1.1us), split
  max    acc = max(t, t')  DVE tensor_tensor bf16 (0.6us)
  gemv   g = acc * xb_i    DVE tensor_tensor bf16 (0.6us)
         z_i = sum(g)      ACT Copy with accum_out (1.1us)
  relu   s_i = relu(z_i)   ACT (0.19us)
  xb_i   [1,D] -> [128,D]  DMA broadcast from DRAM (step-0 partition AP)
PRECISION="bf16" keeps z/scales in f32 but params/candidates/x in bf16
(rel err ~3e-3); "f32" is exact (~1e-6) but ~2x slower (no 2x DVE mode,
and x broadcasts go through PE->PSUM instead of SBUF bf16 tiles).
"""

import numpy as np

N_CORES = 8
D = 1024
DC = D // N_CORES  # 128 columns per core

PRECISION = "bf16"  # "bf16" | "f32"
MULT_ACT_FRAC = {"bf16": 0.30, "f32": 0.70}  # fraction of mults routed to ACT

_CACHE = {}


def _schedule(children_idx, children_dep, children_mask):
    """Prune to the root's ancestor cone and build the edge schedule."""
    n = children_idx.shape[0]
    root = n - 1
    ci = np.asarray(children_idx, dtype=np.int64)
    cd = np.asarray(children_dep, dtype=np.int64)
    cm = np.asarray(children_mask, dtype=bool)

    needed = set()
    stack = [root]
    while stack:
        i = stack.pop()
        if i in needed:
            continue
        needed.add(i)
        for c in range(ci.shape[1]):
            if cm[i, c]:
                stack.append(int(ci[i, c]))

    order = sorted(needed)  # ascending index == topological (children first)
    internal, leaves = [], []
    edges = {}
    for i in order:
        if not cm[i].any():
            leaves.append(i)
            continue
        internal.append(i)
        seen = set()
        elist = []
        for c in range(ci.shape[1]):
            if cm[i, c]:
                key = (int(ci[i, c]), int(cd[i, c]))
                if key not in seen:  # duplicate (child, dep) can't change max
                    seen.add(key)
                    elist.append(key)
        edges[i] = elist

    labels = []  # global label ids, in first-use order
    lab2slot = {}
    for i in internal:
        for _, d in edges[i]:
            if d not in lab2slot:
                lab2slot[d] = len(labels)
                labels.append(d)

    return {
        "root": root,
        "order": order,
        "internal": internal,
        "leaves": leaves,
        "edges": edges,
        "labels": labels,
        "lab2slot": lab2slot,
    }


def _legalize_single_wait(nc):
    """Split multi-wait instructions: this walrus allows 1 sync wait/inst.

    Extra waits move to single-wait InstNoOps inserted just before the
    instruction on the same engine queue (per-engine program order is
    preserved, so the AND-semantics of the wait list is unchanged).
    """
    from concourse import mybir

    for bb in nc.main_func.blocks:
        new_list = []
        for inst in bb.instructions:
            si = inst.sync_info
            if si is not None and si.on_wait and len(si.on_wait) > 1:
                waits = list(si.on_wait)
                for w in waits[:-1]:
                    nop = mybir.InstNoOp(
                        name=nc.get_next_instruction_name(), ins=[], outs=[]
                    )
                    nop.engine = inst.engine
                    nop.sync_info = mybir.SyncInfo(on_wait=[w], on_update=[])
                    new_list.append(nop)
                inst.sync_info = mybir.SyncInfo(
                    on_wait=[waits[-1]], on_update=list(si.on_update)
                )
            new_list.append(inst)
        bb.instructions = new_list


def _build_program(sched, precision, legalize=True):
    import concourse.bass as bass
    import concourse.tile as tile
    from concourse import mybir

    f32 = mybir.dt.float32
    wdt = mybir.dt.bfloat16 if precision == "bf16" else f32
    MUL = mybir.AluOpType.mult
    MAX = mybir.AluOpType.max
    COPY = mybir.ActivationFunctionType.Copy
    RELU = mybir.ActivationFunctionType.Relu

    internal = sched["internal"]
    leaves = sched["leaves"]
    edges = sched["edges"]
    labels = sched["labels"]
    lab2slot = sched["lab2slot"]
    root = sched["root"]
    iloc = {node: t for t, node in enumerate(internal)}

    n_internal = len(internal)
    n_leaves = max(len(leaves), 1)
    n_labels = len(labels)
    frac = MULT_ACT_FRAC[precision]

    nc = bass.Bass()
    pt = nc.dram_tensor("pt", [n_labels, DC, D], wdt, kind="ExternalInput")
    xr = nc.dram_tensor("xr", [n_internal, D], wdt, kind="ExternalInput")
    rl = nc.dram_tensor("rl", [DC, n_leaves], f32, kind="ExternalInput")
    zr = nc.dram_tensor("zr", [DC, 1], f32, kind="ExternalOutput")

    with tile.TileContext(nc) as tc:
        with (
            tc.tile_pool(name="pparams", bufs=1) as ppool,
            tc.tile_pool(name="pwork", bufs=3) as wpool,
            tc.tile_pool(name="psmall", bufs=1) as spool,
            tc.tile_pool(name="ppsum", bufs=2, space="PSUM") as psum_pool,
        ):
            rl_t = spool.tile([DC, n_leaves], f32, tag="rl", name="rl_t")
            nc.sync.dma_start(out=rl_t, in_=rl[:, :])

            if precision == "f32":
                ones = spool.tile([1, DC], f32, tag="ones", name="ones")
                nc.vector.memset(ones, 1.0)

            # param + x-broadcast DMAs, interleaved in first-use order
            pt_t = {}
            xb_t = {}
            for i in internal:
                if precision == "bf16":
                    t = iloc[i]
                    xb = ppool.tile([DC, D], wdt, tag=f"xb{t}", name=f"xb{t}")
                    src = xr[t : t + 1, :]
                    bsrc = bass.AP(
                        tensor=src.tensor,
                        offset=src.offset,
                        ap=[[0, DC]] + list(src.ap)[1:],
                    )
                    nc.sync.dma_start(out=xb, in_=bsrc)
                    xb_t[i] = xb
                for _, d in edges[i]:
                    s = lab2slot[d]
                    if s not in pt_t:
                        p = ppool.tile([DC, D], wdt, tag=f"p{s}", name=f"p{s}")
                        nc.sync.dma_start(out=p, in_=pt[s])
                        pt_t[s] = p

            rel = {}
            for li, leaf in enumerate(leaves):
                rel[leaf] = rl_t[:, li : li + 1]

            n_mult = 0
            n_act = 0

            def mult_into(out_ap, p_ap, s_ap):
                # balance scale-mults between ACT (slow but parallel) and DVE
                nonlocal n_mult, n_act
                n_mult += 1
                if n_act < frac * n_mult:
                    n_act += 1
                    nc.scalar.mul(out_ap, p_ap, s_ap)
                else:
                    nc.vector.tensor_scalar_mul(out_ap, p_ap, s_ap)

            z_root = None
            for i in internal:
                elist = edges[i]
                k = len(elist)
                acc = wpool.tile([DC, D], wdt, tag="acc", name="acc")
                if k == 1:
                    c0, d0 = elist[0]
                    mult_into(acc, pt_t[lab2slot[d0]], rel[c0])
                else:
                    ts = []
                    for c, d in elist:
                        t = wpool.tile([DC, D], wdt, tag="t", name="t", bufs=5)
                        mult_into(t, pt_t[lab2slot[d]], rel[c])
                        ts.append(t)
                    if k == 4:  # pairwise tree: shorter dependency chain
                        ty = wpool.tile([DC, D], wdt, tag="ty", name="ty")
                        nc.vector.tensor_tensor(out=acc, in0=ts[0], in1=ts[1], op=MAX)
                        nc.vector.tensor_tensor(out=ty, in0=ts[2], in1=ts[3], op=MAX)
                        nc.vector.tensor_tensor(out=acc, in0=acc, in1=ty, op=MAX)
                    else:
                        nc.vector.tensor_tensor(out=acc, in0=ts[0], in1=ts[1], op=MAX)
                        for t in ts[2:]:
                            nc.vector.tensor_tensor(out=acc, in0=acc, in1=t, op=MAX)

                zt = spool.tile([DC, 1], f32, tag=f"z{i}", name=f"z{i}")
                if precision == "bf16":
                    g = wpool.tile([DC, D], wdt, tag="g", name="g")
                    nc.vector.tensor_tensor(out=g, in0=acc, in1=xb_t[i], op=MUL)
                    scr = wpool.tile([DC, D], wdt, tag="scr", name="scr", bufs=2)
                    nc.scalar.activation(scr, g, COPY, accum_out=zt)
                else:
                    t = iloc[i]
                    xst = wpool.tile([1, D], f32, tag="xst", name="xst", bufs=4)
                    nc.sync.dma_start(out=xst, in_=xr[t : t + 1, :])
                    xbp = psum_pool.tile([DC, D], f32, tag="xbp", name="xbp")
                    nc.tensor.matmul(xbp[:, 0:512], ones, xst[:, 0:512])
                    nc.tensor.matmul(xbp[:, 512:D], ones, xst[:, 512:D])
                    scr = wpool.tile([DC, D], f32, tag="scr", name="scr", bufs=2)
                    nc.vector.scalar_tensor_tensor(
                        out=scr, in0=acc, scalar=1.0, in1=xbp,
                        op0=MUL, op1=MUL, accum_out=zt,
                    )

                if i == root:
                    z_root = zt
                else:
                    rt = spool.tile([DC, 1], f32, tag=f"r{i}", name=f"r{i}")
                    nc.scalar.activation(rt, zt, RELU)
                    rel[i] = rt

            nc.sync.dma_start(out=zr[:, :], in_=z_root)

    if legalize:
        _legalize_single_wait(nc)
    return nc


def _prepare(embeddings, params, children_idx, children_dep, children_mask,
             legalize=True):
    import ml_dtypes

    emb = np.ascontiguousarray(np.asarray(embeddings, dtype=np.float32))
    par = np.asarray(params, dtype=np.float32)
    sched = _schedule(children_idx, children_dep, children_mask)

    key = (
        PRECISION,
        legalize,
        tuple(sched["order"]),
        tuple(sched["labels"]),
        tuple((i, tuple(e)) for i, e in sched["edges"].items()),
    )
    if key in _CACHE:
        nc = _CACHE[key]
    else:
        nc = _build_program(sched, PRECISION, legalize=legalize)
        _CACHE[key] = nc

    wnp = ml_dtypes.bfloat16 if PRECISION == "bf16" else np.float32
    internal = sched["internal"]
    leaves = sched["leaves"]
    labels = sched["labels"]
    n_leaves = max(len(leaves), 1)

    xr = np.ascontiguousarray(emb[internal]).astype(wnp)
    p_used = par[labels]  # [L, D, D]
    in_maps = []
    for k in range(N_CORES):
        cols = slice(k * DC, (k + 1) * DC)
        pt_k = np.ascontiguousarray(
            p_used[:, :, cols].transpose(0, 2, 1)
        ).astype(wnp)
        rl_k = np.zeros((DC, n_leaves), dtype=np.float32)
        if leaves:
            rl_k[:, : len(leaves)] = np.maximum(emb[leaves][:, cols], 0.0).T
        in_maps.append({"pt": pt_k, "xr": xr, "rl": rl_k})
    return sched, nc, in_maps


def _run(embeddings, params, children_idx, children_dep, children_mask,
         trace=False):
    emb = np.asarray(embeddings, dtype=np.float32)
    cm = np.asarray(children_mask, dtype=bool)
    root = emb.shape[0] - 1
    if not cm[root].any():  # degenerate: root is a leaf
        return emb[root : root + 1].copy(), None

    from concourse.bass_utils import run_bass_kernel_spmd

    sched, nc, in_maps = _prepare(
        embeddings, params, children_idx, children_dep, children_mask
    )
    bkr = run_bass_kernel_spmd(
        nc, in_maps, core_ids=list(range(N_CORES)), trace=trace
    )
    out = np.concatenate(
        [bkr.results[k]["zr"].reshape(DC) for k in range(N_CORES)]
    ).reshape(1, D)
    return out.astype(np.float32), bkr


def kernel(embeddings, params, children_idx, children_dep, children_mask):
    out, _ = _run(embeddings, params, children_idx, children_dep, children_mask)
    return out


def run_traced(embeddings, params, children_idx, children_dep, children_mask):
    return _run(
        embeddings, params, children_idx, children_dep, children_mask, trace=True
    )

